# revision 29
# baseline (speedup 1.0000x reference)
"""Trainium2 Bass kernel for nn_GAT_RL (GAT message passing, B=128, N=64).

Self-contained: hardcodes shapes/sharding. Shards batch 128 -> 8 cores x 16.

Math (per sample):
  r_emb = MLP(robot 9->64->128, relu both), h_emb = MLP(humans 5->64->128)
  X = concat -> [64, 128]
  GAT layer: score[i,j] = w2^T relu(W1a^T x_i + W1b^T x_j + b1) + b2
             (decomposition of MLP([x_i, x_j]) first layer)
  e = LeakyReLU(score, 0.04); mask col 0 for rows i>0; softmax over j
  H = attn @ X;  out = H1 + H2 + X
"""

import os
import sys

sys.path.insert(0, "/opt/trn_rl_repo")

import numpy as np

import concourse.bass as bass
import concourse.bacc as bacc_mod
import concourse.mybir as mybir
import concourse.tile as tile
from concourse.bass_utils import run_bass_kernel_spmd

F32 = mybir.dt.float32
F32R = mybir.dt.float32r
AF = mybir.ActivationFunctionType
ALU = mybir.AluOpType
AX = mybir.AxisListType

N_CORES = 8
B = 128
BC = B // N_CORES  # 16 samples per core
N_R, N_H, RD, HD = 1, 63, 9, 5
N = N_R + N_H  # 64
D = 128  # X_DIM
HID = 64
NEG_SLOPE = 0.04
MASK_VAL = -9.0e15
NPAIR = BC // 2  # sample pairs per core


def r32(ap):
    # float32r (1 cyc/row) requires fp32r-rounded producers; plain fp32 for now
    return ap


def _mask_np():
    """Additive adjacency mask in per-pair score layout [i, (s', j)].

    Mask (add MASK_VAL) where j == 0 and i > 0 (humans don't see robot).
    """
    m = np.zeros((64, 128), np.float32)
    m[1:, 0] = MASK_VAL
    m[1:, 64] = MASK_VAL
    return m


def build_bass(debug=False, stage=99):
    nc = bacc_mod.Bacc(trn_type="TRN2")

    # ---- I/O ----
    robot = nc.dram_tensor("robot_state", [BC, N_R, RD], F32, kind="ExternalInput")
    humans = nc.dram_tensor("human_states", [BC, N_H, HD], F32, kind="ExternalInput")
    w = {}
    for nm, shp in [
        ("wr_W1", [RD, HID]), ("wr_b1", [HID]), ("wr_W2", [HID, D]), ("wr_b2", [D]),
        ("wh_W1", [HD, HID]), ("wh_b1", [HID]), ("wh_W2", [HID, D]), ("wh_b2", [D]),
        ("g0_W1", [2 * D, 2 * D]), ("g0_b1", [2 * D]), ("g0_W2", [2 * D, 1]), ("g0_b2", [1]),
        ("g1_W1", [2 * D, 2 * D]), ("g1_b1", [2 * D]), ("g1_W2", [2 * D, 1]), ("g1_b2", [1]),
    ]:
        w[nm] = nc.dram_tensor(nm, shp, F32, kind="ExternalInput")
    out_dram = nc.dram_tensor("out", [BC, N, D], F32, kind="ExternalOutput")

    dbg = {}
    if debug:
        for nm, shp in [("dbg_xt", [128, 1024]), ("dbg_h1t", [128, 1024]),
                        ("dbg_utb", [128, 2048]), ("dbg_vt", [128, 2048]),
                        ("dbg_att", [64, 128]), ("dbg_score", [64, 128])]:
            dbg[nm] = nc.dram_tensor(nm, shp, F32, kind="ExternalOutput")

    ident_dram = nc.inline_tensor(np.eye(128, dtype=np.float32), name="ident128")
    mask_dram = nc.inline_tensor(_mask_np(), name="adjmask")
    ones_dram = nc.inline_tensor(np.ones((1, 1024), np.float32), name="ones1024")

    with tile.TileContext(nc) as tc, \
         tc.tile_pool(name="const", bufs=1) as constp, \
         tc.tile_pool(name="big", bufs=1) as bigp, \
         tc.tile_pool(name="uv", bufs=2) as uvp, \
         tc.tile_pool(name="tbuf", bufs=3) as tbufp, \
         tc.tile_pool(name="small", bufs=4) as smallp, \
         tc.tile_pool(name="score_ps", bufs=2, space="PSUM") as score_psp, \
         tc.tile_pool(name="uv_ps", bufs=2, space="PSUM") as uv_psp, \
         tc.tile_pool(name="tr_ps", bufs=4, space="PSUM") as tr_psp:

        # ---------- constants ----------
        ident = constp.tile([128, 128], F32, tag="ident")
        nc.sync.dma_start(ident[:], ident_dram[:])
        ones_sb = constp.tile([1, 128], F32, tag="ones_sb")
        nc.sync.dma_start(ones_sb[:], ones_dram[:, 0:128])
        maskt = constp.tile([64, 128], F32, tag="mask")
        nc.sync.dma_start(maskt[:], mask_dram[:])

        def pe_t(dst_sb, src_ap, copy_eng="scalar"):
            """PE-transpose src [P,F] -> psum [F,P], then copy into dst_sb."""
            p, f = src_ap.shape[0], src_ap.free_size()
            ps = tr_psp.tile([128, 128], F32, tag="trps")
            nc.tensor.transpose(ps[0:f, 0:p], src_ap, ident[0:p, 0:p])
            eng = nc.scalar if copy_eng == "scalar" else nc.vector
            if copy_eng == "scalar":
                eng.copy(dst_sb, ps[0:f, 0:p])
            else:
                eng.tensor_copy(dst_sb, ps[0:f, 0:p])

        # ---------- embed weights (bias folded as extra lhsT row) ----------
        wr1e = constp.tile([RD + 1, HID], F32, tag="wr1e")
        nc.sync.dma_start(wr1e[0:RD, :], w["wr_W1"][:, :])
        nc.sync.dma_start(wr1e[RD:RD + 1, :], w["wr_b1"][:].unsqueeze(0))
        wr2e = constp.tile([HID + 1, D], F32, tag="wr2e")
        nc.sync.dma_start(wr2e[0:HID, :], w["wr_W2"][:, :])
        nc.sync.dma_start(wr2e[HID:HID + 1, :], w["wr_b2"][:].unsqueeze(0))
        wh1e = constp.tile([HD + 1, HID], F32, tag="wh1e")
        nc.sync.dma_start(wh1e[0:HD, :], w["wh_W1"][:, :])
        nc.sync.dma_start(wh1e[HD:HD + 1, :], w["wh_b1"][:].unsqueeze(0))
        wh2e = constp.tile([HID + 1, D], F32, tag="wh2e")
        nc.sync.dma_start(wh2e[0:HID, :], w["wh_W2"][:, :])
        nc.sync.dma_start(wh2e[HID:HID + 1, :], w["wh_b2"][:].unsqueeze(0))

        # ---------- GAT weights ----------
        gw = []
        for li, pre in enumerate(("g0", "g1")):
            w1t = constp.tile([128, 256], F32, tag=f"w1t{li}")  # rows 0..127 of W1
            nc.sync.dma_start(w1t[:], w[f"{pre}_W1"][0:128, :])
            w1b = constp.tile([128, 256], F32, tag=f"w1b{li}")  # rows 128..255
            nc.sync.dma_start(w1b[:], w[f"{pre}_W1"][128:256, :])
            brow = constp.tile([1, 256], F32, tag=f"b1row{li}")
            nc.sync.dma_start(brow[:], w[f"{pre}_b1"][:].unsqueeze(0))
            b1t = constp.tile([128, 2], F32, tag=f"b1t{li}")
            pe_t(b1t[:, 0:1], brow[0:1, 0:128])
            pe_t(b1t[:, 1:2], brow[0:1, 128:256])
            w2row = constp.tile([1, 256], F32, tag=f"w2row{li}")
            nc.sync.dma_start(w2row[:], w[f"{pre}_W2"][:, 0].unsqueeze(0))
            w2t = constp.tile([128, 2], F32, tag=f"w2t{li}")
            pe_t(w2t[:, 0:1], w2row[0:1, 0:128])
            pe_t(w2t[:, 1:2], w2row[0:1, 128:256])
            b2s = constp.tile([1, 1], F32, tag=f"b2s{li}")
            nc.sync.dma_start(b2s[:], w[f"{pre}_b2"][:].unsqueeze(0))
            b2bc = constp.tile([128, 1], F32, tag=f"b2bc{li}")
            ps = tr_psp.tile([128, 128], F32, tag="trps")
            nc.tensor.matmul(ps[0:128, 0:1], ones_sb[0:1, :], b2s[0:1, 0:1])
            nc.scalar.copy(b2bc[:], ps[0:128, 0:1])
            gw.append(dict(w1t=w1t, w1b=w1b, b1t=b1t, w2t=w2t, b2bc=b2bc))

        # ---------- embed: humans ----------
        # hsT_ext [6, 1008]: rows 0..4 = human_states^T (d, s*63+n), row 5 = 1
        hsT = constp.tile([HD + 1, BC * N_H], F32, tag="hsT")
        hs_flat = humans[:, :, :].rearrange("s n d -> (s n) d")  # [1008, 5]
        for t in range(8):
            rows = hs_flat[126 * t:126 * (t + 1), :]  # [126, 5] contiguous
            rt = smallp.tile([126, HD], F32, tag="hsrows")
            nc.sync.dma_start(rt[:], rows)
            pe_t(hsT[0:HD, 126 * t:126 * (t + 1)], rt[:])
        nc.sync.dma_start(hsT[HD:HD + 1, :], ones_dram[:, 0:BC * N_H])

        # h1hT_ext [65, 1008] = relu(wh1e^T @ hsT_ext) ; row 64 = 1
        h1hT = constp.tile([HID + 1, BC * N_H], F32, tag="h1hT")
        for ch in range(2):
            cs = 504 * ch
            ps = uv_psp.tile([128, 512], F32, tag="uvps")
            nc.tensor.matmul(ps[0:HID, 0:504], r32(wh1e[:]), r32(hsT[:, cs:cs + 504]))
            nc.scalar.activation(h1hT[0:HID, cs:cs + 504], ps[0:HID, 0:504], AF.Relu)
        nc.sync.dma_start(h1hT[HID:HID + 1, :], ones_dram[:, 0:BC * N_H])

        # XT_all [128, 1024]: col 64*s + node ; node0=robot, 1..63=humans
        xt_all = bigp.tile([128, BC * N], F32, tag="xt")
        for ch in range(2):
            cs = 504 * ch
            ps = uv_psp.tile([128, 512], F32, tag="uvps")
            nc.tensor.matmul(ps[:, 0:504], r32(wh2e[:]), r32(h1hT[:, cs:cs + 504]))
            dst = xt_all[:].rearrange("p (s n) -> p s n", s=BC)[:, 8 * ch:8 * ch + 8, 1:64]
            nc.scalar.activation(dst, ps[:, 0:504].rearrange("p (s n) -> p s n", s=8), AF.Relu)

        # ---------- embed: robot ----------
        rsb = smallp.tile([BC, RD], F32, tag="rsb")
        nc.sync.dma_start(rsb[:], robot[:, 0, :])
        rT = constp.tile([RD + 1, BC], F32, tag="rT")
        pe_t(rT[0:RD, :], rsb[:])
        nc.sync.dma_start(rT[RD:RD + 1, :], ones_dram[:, 0:BC])
        h1rT = constp.tile([HID + 1, BC], F32, tag="h1rT")
        ps = tr_psp.tile([128, 128], F32, tag="trps")
        nc.tensor.matmul(ps[0:HID, 0:BC], wr1e[:], rT[:])
        nc.scalar.activation(h1rT[0:HID, :], ps[0:HID, 0:BC], AF.Relu)
        nc.sync.dma_start(h1rT[HID:HID + 1, :], ones_dram[:, 0:BC])
        ps = tr_psp.tile([128, 128], F32, tag="trps")
        nc.tensor.matmul(ps[:, 0:BC], wr2e[:], h1rT[:])
        nc.scalar.activation(
            xt_all[:].rearrange("p (s n) -> p s n", s=BC)[:, :, 0:1],
            ps[:, 0:BC].unsqueeze(2), AF.Relu)

        if debug:
            nc.sync.dma_start(dbg["dbg_xt"][:], xt_all[:])

        # ---------- GAT layers ----------
        h1t_all = bigp.tile([128, BC * N], F32, tag="h1t")
        xh1_all = bigp.tile([128, BC * N], F32, tag="xh1")

        for li in range(2 if stage >= 99 else (1 if stage >= 2 else 0)):
            g = gw[li]
            xlt = xt_all if li == 0 else h1t_all

            # UV: UTb_all/VT_all [128, 2048] col = 1024*h + 64*s + i
            utb = uvp.tile([128, 2048], F32, tag="utb")
            vt = uvp.tile([128, 2048], F32, tag="vt")
            for h in range(2):
                for ch in range(2):
                    cs = 512 * ch
                    ps = uv_psp.tile([128, 512], F32, tag="uvps")
                    nc.tensor.matmul(ps[:], r32(g["w1t"][:, 128 * h:128 * (h + 1)]),
                                     r32(xlt[:, cs:cs + 512]))
                    nc.scalar.activation(utb[:, 1024 * h + cs:1024 * h + cs + 512],
                                         ps[:], AF.Identity, bias=g["b1t"][:, h:h + 1])
                for ch in range(2):
                    cs = 512 * ch
                    ps = uv_psp.tile([128, 512], F32, tag="uvps")
                    nc.tensor.matmul(ps[:], r32(g["w1b"][:, 128 * h:128 * (h + 1)]),
                                     r32(xlt[:, cs:cs + 512]))
                    nc.scalar.copy(vt[:, 1024 * h + cs:1024 * h + cs + 512], ps[:])

            if debug and li == 0:
                nc.sync.dma_start(dbg["dbg_utb"][:], utb[:])
                nc.sync.dma_start(dbg["dbg_vt"][:], vt[:])

            # per sample-pair q: add + relu + reduce + softmax + attn@X
            for q in range(NPAIR if stage >= 60 else (1 if stage >= 3 else 0)):
                # T free layout (s', j, i): flat = 4096 s' + 64 j + i
                tt = []
                for h in range(2):
                    t = tbufp.tile([128, 8192], F32, tag="T")
                    o4 = t[:].rearrange("p (s j i) -> p s j i", s=2, j=64)
                    vsl = vt[:, 1024 * h + 128 * q:1024 * h + 128 * q + 128]
                    usl = utb[:, 1024 * h + 128 * q:1024 * h + 128 * q + 128]
                    v4 = vsl.rearrange("p (s j) -> p s j", s=2).unsqueeze(3) \
                        .broadcast_to((128, 2, 64, 64))
                    u4 = usl.rearrange("p (s i) -> p s i", s=2).unsqueeze(2) \
                        .broadcast_to((128, 2, 64, 64))
                    nc.vector.tensor_tensor(o4, v4, u4, op=ALU.add)
                    nc.scalar.activation(t[:], t[:], AF.Relu)
                    tt.append(t)
                # scores [64(i), 128(s',j)]: T-chunk stationary, w2 moving
                score = score_psp.tile([64, 128], F32, tag="score")
                for h in range(2):
                    for c in range(128):
                        # PSUM zero-region: start only once per bank/tile;
                        # later writes land on pending-zero bytes (overwrite),
                        # second-half writes accumulate.
                        nc.tensor.matmul(
                            score[0:64, c:c + 1],
                            r32(tt[h][:, 64 * c:64 * (c + 1)]),
                            r32(g["w2t"][:, h:h + 1]),
                            start=(h == 0 and c == 0), stop=(h == 1 and c == 127),
                            skip_group_check=True)

                if debug and li == 0 and q == 0:
                    ssb = smallp.tile([64, 128], F32, tag="ssb")
                    nc.vector.tensor_copy(ssb[:], score[:])
                    nc.sync.dma_start(dbg["dbg_score"][:], ssb[:])

                if stage < 4:
                    continue
                # LeakyReLU(s + b2), +mask, softmax over j per (i, s')
                att = smallp.tile([64, 128], F32, tag="att")
                nc.scalar.activation(att[:], score[:], AF.Identity,
                                     bias=g["b2bc"][0:64, 0:1])
                # LeakyReLU(x) = max(0.04*x, x)
                nc.vector.scalar_tensor_tensor(att[:], att[:], NEG_SLOPE, att[:],
                                               op0=ALU.mult, op1=ALU.max)
                nc.vector.tensor_tensor(att[:], att[:], maskt[:], op=ALU.add)
                a3 = att[:].rearrange("p (s j) -> p s j", s=2)
                mx = smallp.tile([64, 2], F32, tag="mx")
                nc.vector.tensor_reduce(mx[:], a3, axis=AX.X, op=ALU.max)
                nc.vector.tensor_tensor(
                    a3, a3, mx[:].unsqueeze(2).broadcast_to((64, 2, 64)),
                    op=ALU.subtract)
                nc.scalar.activation(att[:], att[:], AF.Exp)
                sm = smallp.tile([64, 2], F32, tag="sm")
                nc.vector.tensor_reduce(sm[:], a3, axis=AX.X, op=ALU.add)
                nc.vector.reciprocal(sm[:], sm[:])
                nc.vector.tensor_tensor(
                    a3, a3, sm[:].unsqueeze(2).broadcast_to((64, 2, 64)),
                    op=ALU.mult)

                if debug and li == 0 and q == 0:
                    nc.sync.dma_start(dbg["dbg_att"][:], att[:])

                if stage < 5:
                    continue
                # H^T [d, (s',i)] per sample; all matmul operands at
                # base partition 0 (offset-64 operands crash HW)
                hps = tr_psp.tile([128, 128], F32, tag="trps")
                for sp in range(2):
                    atT = smallp.tile([64, 64], F32, tag="atT")
                    ps = tr_psp.tile([128, 128], F32, tag="trps")
                    nc.tensor.transpose(ps[0:64, 0:64],
                                        att[:, 64 * sp:64 * sp + 64],
                                        ident[0:64, 0:64])
                    nc.vector.tensor_copy(atT[:], ps[0:64, 0:64])
                    xl = smallp.tile([64, 128], F32, tag="xl")
                    ps2 = tr_psp.tile([128, 128], F32, tag="trps")
                    nc.tensor.transpose(
                        ps2[0:64, 0:128],
                        xlt[:, 128 * q + 64 * sp:128 * q + 64 * sp + 64],
                        ident[:])
                    nc.vector.tensor_copy(xl[:], ps2[0:64, 0:128])
                    nc.tensor.matmul(hps[:, 64 * sp:64 * sp + 64],
                                     xl[:], atT[:],
                                     start=(sp == 0), stop=(sp == 1),
                                     skip_group_check=True)
                if li == 0:
                    nc.scalar.copy(h1t_all[:, 128 * q:128 * (q + 1)], hps[:])
                else:
                    outp = smallp.tile([128, 128], F32, tag="outp")
                    nc.vector.tensor_tensor(outp[:], hps[:],
                                            xh1_all[:, 128 * q:128 * (q + 1)], op=ALU.add)
                    ps2 = tr_psp.tile([128, 128], F32, tag="trps")
                    nc.tensor.transpose(ps2[:], outp[:], ident[:])
                    orows = smallp.tile([128, 128], F32, tag="orows")
                    nc.scalar.copy(orows[:], ps2[:])
                    nc.sync.dma_start(
                        out_dram[2 * q:2 * q + 2, :, :].rearrange("s n d -> (s n) d"),
                        orows[:])

            if li == 0 and stage >= 60:
                nc.vector.tensor_tensor(xh1_all[:], xt_all[:], h1t_all[:], op=ALU.add)
                if debug:
                    nc.sync.dma_start(dbg["dbg_h1t"][:], h1t_all[:])

    nc.compile()
    nc.finalize()
    return nc


_NC_CACHE = {}


def get_nc():
    if "nc" not in _NC_CACHE:
        _NC_CACHE["nc"] = build_bass()
    return _NC_CACHE["nc"]


def shard_inputs(inputs):
    in_maps = []
    for c in range(N_CORES):
        m = {}
        for k, v in inputs.items():
            v = np.asarray(v, dtype=np.float32)
            if k in ("robot_state", "human_states"):
                m[k] = np.ascontiguousarray(v[c * BC:(c + 1) * BC])
            else:
                m[k] = v
        in_maps.append(m)
    return in_maps


def run_sharded(inputs, trace=False, **kw):
    nc = get_nc()
    br = run_bass_kernel_spmd(nc, shard_inputs(inputs),
                              list(range(N_CORES)), trace=trace, **kw)
    out = np.concatenate([br.results[c]["out"] for c in range(N_CORES)], axis=0)
    return out, br


def kernel(**inputs):
    out, _ = run_sharded(inputs, trace=False)
    return out


# revision 30
# speedup vs baseline: 2.3127x; 2.3127x over previous
"""Trainium2 Bass kernel for nn_GAT_RL (GAT message passing, B=128, N=64).

Self-contained: hardcodes shapes/sharding. Shards batch 128 -> 8 cores x 16.

Math (per sample):
  r_emb = MLP(robot 9->64->128, relu both), h_emb = MLP(humans 5->64->128)
  X = concat -> [64, 128]
  GAT layer: score[i,j] = w2^T relu(W1a^T x_i + W1b^T x_j + b1) + b2
             (decomposition of MLP([x_i, x_j]) first layer)
  e = LeakyReLU(score, 0.04); mask col 0 for rows i>0; softmax over j
  H = attn @ X;  out = H1 + H2 + X
"""

import os
import sys

sys.path.insert(0, "/opt/trn_rl_repo")

import numpy as np

import concourse.bass as bass
import concourse.bacc as bacc_mod
import concourse.mybir as mybir
import concourse.tile as tile
from concourse.bass_utils import run_bass_kernel_spmd

F32 = mybir.dt.float32
F32R = mybir.dt.float32r
BF16 = mybir.dt.bfloat16
AF = mybir.ActivationFunctionType
ALU = mybir.AluOpType
AX = mybir.AxisListType

N_CORES = 8
B = 128
BC = B // N_CORES  # 16 samples per core
N_R, N_H, RD, HD = 1, 63, 9, 5
N = N_R + N_H  # 64
D = 128  # X_DIM
HID = 64
NEG_SLOPE = 0.04
MASK_VAL = -9.0e15
NPAIR = BC // 2  # sample pairs per core


def r32(ap):
    # float32r (1 cyc/row) requires fp32r-rounded producers; plain fp32 for now
    return ap


def _mask_np():
    """Additive adjacency mask in per-pair score layout [i, (s', j)].

    Mask (add MASK_VAL) where j == 0 and i > 0 (humans don't see robot).
    """
    m = np.zeros((64, 128), np.float32)
    m[1:, 0] = MASK_VAL
    m[1:, 64] = MASK_VAL
    return m


def build_bass(debug=False, stage=99):
    nc = bacc_mod.Bacc(trn_type="TRN2")

    # ---- I/O ----
    robot = nc.dram_tensor("robot_state", [BC, N_R, RD], F32, kind="ExternalInput")
    humans = nc.dram_tensor("human_states", [BC, N_H, HD], F32, kind="ExternalInput")
    w = {}
    for nm, shp in [
        ("wr_W1", [RD, HID]), ("wr_b1", [HID]), ("wr_W2", [HID, D]), ("wr_b2", [D]),
        ("wh_W1", [HD, HID]), ("wh_b1", [HID]), ("wh_W2", [HID, D]), ("wh_b2", [D]),
        ("g0_W1", [2 * D, 2 * D]), ("g0_b1", [2 * D]), ("g0_W2", [2 * D, 1]), ("g0_b2", [1]),
        ("g1_W1", [2 * D, 2 * D]), ("g1_b1", [2 * D]), ("g1_W2", [2 * D, 1]), ("g1_b2", [1]),
    ]:
        w[nm] = nc.dram_tensor(nm, shp, F32, kind="ExternalInput")
    out_dram = nc.dram_tensor("out", [BC, N, D], F32, kind="ExternalOutput")

    dbg = {}
    if debug:
        for nm, shp in [("dbg_xt", [128, 1024]), ("dbg_h1t", [128, 1024]),
                        ("dbg_utb", [128, 2048]), ("dbg_vt", [128, 2048]),
                        ("dbg_att", [64, 128]), ("dbg_score", [64, 128])]:
            dbg[nm] = nc.dram_tensor(nm, shp, F32, kind="ExternalOutput")

    ident_dram = nc.inline_tensor(np.eye(128, dtype=np.float32), name="ident128")
    mask_dram = nc.inline_tensor(_mask_np(), name="adjmask")
    ones_dram = nc.inline_tensor(np.ones((1, 1024), np.float32), name="ones1024")

    with tile.TileContext(nc) as tc, \
         tc.tile_pool(name="const", bufs=1) as constp, \
         tc.tile_pool(name="big", bufs=1) as bigp, \
         tc.tile_pool(name="uv", bufs=2) as uvp, \
         tc.tile_pool(name="tbuf", bufs=3) as tbufp, \
         tc.tile_pool(name="small", bufs=4) as smallp, \
         tc.tile_pool(name="score_ps", bufs=2, space="PSUM") as score_psp, \
         tc.tile_pool(name="uv_ps", bufs=2, space="PSUM") as uv_psp, \
         tc.tile_pool(name="tr_ps", bufs=4, space="PSUM") as tr_psp:

        # ---------- constants ----------
        ident = constp.tile([128, 128], F32, tag="ident")
        nc.sync.dma_start(ident[:], ident_dram[:])
        ones_sb = constp.tile([1, 128], F32, tag="ones_sb")
        nc.sync.dma_start(ones_sb[:], ones_dram[:, 0:128])
        maskt = constp.tile([64, 128], F32, tag="mask")
        nc.sync.dma_start(maskt[:], mask_dram[:])

        def pe_t(dst_sb, src_ap, copy_eng="scalar"):
            """PE-transpose src [P,F] -> psum [F,P], then copy into dst_sb."""
            p, f = src_ap.shape[0], src_ap.free_size()
            ps = tr_psp.tile([128, 128], F32, tag="trps")
            nc.tensor.transpose(ps[0:f, 0:p], src_ap, ident[0:p, 0:p])
            eng = nc.scalar if copy_eng == "scalar" else nc.vector
            if copy_eng == "scalar":
                eng.copy(dst_sb, ps[0:f, 0:p])
            else:
                eng.tensor_copy(dst_sb, ps[0:f, 0:p])

        # ---------- embed weights (bias folded as extra lhsT row) ----------
        wr1e = constp.tile([RD + 1, HID], F32, tag="wr1e")
        nc.sync.dma_start(wr1e[0:RD, :], w["wr_W1"][:, :])
        nc.sync.dma_start(wr1e[RD:RD + 1, :], w["wr_b1"][:].unsqueeze(0))
        wr2e = constp.tile([HID + 1, D], F32, tag="wr2e")
        nc.sync.dma_start(wr2e[0:HID, :], w["wr_W2"][:, :])
        nc.sync.dma_start(wr2e[HID:HID + 1, :], w["wr_b2"][:].unsqueeze(0))
        wh1e = constp.tile([HD + 1, HID], F32, tag="wh1e")
        nc.sync.dma_start(wh1e[0:HD, :], w["wh_W1"][:, :])
        nc.sync.dma_start(wh1e[HD:HD + 1, :], w["wh_b1"][:].unsqueeze(0))
        wh2e = constp.tile([HID + 1, D], F32, tag="wh2e")
        nc.sync.dma_start(wh2e[0:HID, :], w["wh_W2"][:, :])
        nc.sync.dma_start(wh2e[HID:HID + 1, :], w["wh_b2"][:].unsqueeze(0))

        # ---------- GAT weights ----------
        gw = []
        for li, pre in enumerate(("g0", "g1")):
            w1t = constp.tile([128, 256], F32, tag=f"w1t{li}")  # rows 0..127 of W1
            nc.sync.dma_start(w1t[:], w[f"{pre}_W1"][0:128, :])
            w1b = constp.tile([128, 256], F32, tag=f"w1b{li}")  # rows 128..255
            nc.sync.dma_start(w1b[:], w[f"{pre}_W1"][128:256, :])
            brow = constp.tile([1, 256], F32, tag=f"b1row{li}")
            nc.sync.dma_start(brow[:], w[f"{pre}_b1"][:].unsqueeze(0))
            b1t = constp.tile([128, 2], F32, tag=f"b1t{li}")
            pe_t(b1t[:, 0:1], brow[0:1, 0:128])
            pe_t(b1t[:, 1:2], brow[0:1, 128:256])
            w2row = constp.tile([1, 256], F32, tag=f"w2row{li}")
            nc.sync.dma_start(w2row[:], w[f"{pre}_W2"][:, 0].unsqueeze(0))
            w2t = constp.tile([128, 2], F32, tag=f"w2t{li}")
            pe_t(w2t[:, 0:1], w2row[0:1, 0:128])
            pe_t(w2t[:, 1:2], w2row[0:1, 128:256])
            w2tb = constp.tile([128, 2], BF16, tag=f"w2tb{li}")
            nc.vector.tensor_copy(w2tb[:], w2t[:])
            b2s = constp.tile([1, 1], F32, tag=f"b2s{li}")
            nc.sync.dma_start(b2s[:], w[f"{pre}_b2"][:].unsqueeze(0))
            b2bc = constp.tile([128, 1], F32, tag=f"b2bc{li}")
            ps = tr_psp.tile([128, 128], F32, tag="trps")
            nc.tensor.matmul(ps[0:128, 0:1], ones_sb[0:1, :], b2s[0:1, 0:1])
            nc.scalar.copy(b2bc[:], ps[0:128, 0:1])
            gw.append(dict(w1t=w1t, w1b=w1b, b1t=b1t, w2t=w2t, w2tb=w2tb, b2bc=b2bc))

        # ---------- embed: humans ----------
        # hsT_ext [6, 1008]: rows 0..4 = human_states^T (d, s*63+n), row 5 = 1
        hsT = constp.tile([HD + 1, BC * N_H], F32, tag="hsT")
        hs_flat = humans[:, :, :].rearrange("s n d -> (s n) d")  # [1008, 5]
        for t in range(8):
            rows = hs_flat[126 * t:126 * (t + 1), :]  # [126, 5] contiguous
            rt = smallp.tile([126, HD], F32, tag="hsrows")
            nc.sync.dma_start(rt[:], rows)
            pe_t(hsT[0:HD, 126 * t:126 * (t + 1)], rt[:])
        nc.sync.dma_start(hsT[HD:HD + 1, :], ones_dram[:, 0:BC * N_H])

        # h1hT_ext [65, 1008] = relu(wh1e^T @ hsT_ext) ; row 64 = 1
        h1hT = constp.tile([HID + 1, BC * N_H], F32, tag="h1hT")
        for ch in range(2):
            cs = 504 * ch
            ps = uv_psp.tile([128, 512], F32, tag="uvps")
            nc.tensor.matmul(ps[0:HID, 0:504], r32(wh1e[:]), r32(hsT[:, cs:cs + 504]))
            nc.scalar.activation(h1hT[0:HID, cs:cs + 504], ps[0:HID, 0:504], AF.Relu)
        nc.sync.dma_start(h1hT[HID:HID + 1, :], ones_dram[:, 0:BC * N_H])

        # XT_all [128, 1024]: col 64*s + node ; node0=robot, 1..63=humans
        xt_all = bigp.tile([128, BC * N], F32, tag="xt")
        for ch in range(2):
            cs = 504 * ch
            ps = uv_psp.tile([128, 512], F32, tag="uvps")
            nc.tensor.matmul(ps[:, 0:504], r32(wh2e[:]), r32(h1hT[:, cs:cs + 504]))
            dst = xt_all[:].rearrange("p (s n) -> p s n", s=BC)[:, 8 * ch:8 * ch + 8, 1:64]
            nc.scalar.activation(dst, ps[:, 0:504].rearrange("p (s n) -> p s n", s=8), AF.Relu)

        # ---------- embed: robot ----------
        rsb = smallp.tile([BC, RD], F32, tag="rsb")
        nc.sync.dma_start(rsb[:], robot[:, 0, :])
        rT = constp.tile([RD + 1, BC], F32, tag="rT")
        pe_t(rT[0:RD, :], rsb[:])
        nc.sync.dma_start(rT[RD:RD + 1, :], ones_dram[:, 0:BC])
        h1rT = constp.tile([HID + 1, BC], F32, tag="h1rT")
        ps = tr_psp.tile([128, 128], F32, tag="trps")
        nc.tensor.matmul(ps[0:HID, 0:BC], wr1e[:], rT[:])
        nc.scalar.activation(h1rT[0:HID, :], ps[0:HID, 0:BC], AF.Relu)
        nc.sync.dma_start(h1rT[HID:HID + 1, :], ones_dram[:, 0:BC])
        ps = tr_psp.tile([128, 128], F32, tag="trps")
        nc.tensor.matmul(ps[:, 0:BC], wr2e[:], h1rT[:])
        nc.scalar.activation(
            xt_all[:].rearrange("p (s n) -> p s n", s=BC)[:, :, 0:1],
            ps[:, 0:BC].unsqueeze(2), AF.Relu)

        if debug:
            nc.sync.dma_start(dbg["dbg_xt"][:], xt_all[:])

        # ---------- GAT layers ----------
        h1t_all = bigp.tile([128, BC * N], F32, tag="h1t")
        xh1_all = bigp.tile([128, BC * N], F32, tag="xh1")

        for li in range(2 if stage >= 99 else (1 if stage >= 2 else 0)):
            g = gw[li]
            xlt = xt_all if li == 0 else h1t_all

            # UV: UTb_all/VT_all [128, 2048] col = 1024*h + 64*s + i
            utb = uvp.tile([128, 2048], BF16, tag="utb")
            vt = uvp.tile([128, 2048], BF16, tag="vt")
            for h in range(2):
                for ch in range(2):
                    cs = 512 * ch
                    ps = uv_psp.tile([128, 512], F32, tag="uvps")
                    nc.tensor.matmul(ps[:], r32(g["w1t"][:, 128 * h:128 * (h + 1)]),
                                     r32(xlt[:, cs:cs + 512]))
                    nc.scalar.activation(utb[:, 1024 * h + cs:1024 * h + cs + 512],
                                         ps[:], AF.Identity, bias=g["b1t"][:, h:h + 1])
                for ch in range(2):
                    cs = 512 * ch
                    ps = uv_psp.tile([128, 512], F32, tag="uvps")
                    nc.tensor.matmul(ps[:], r32(g["w1b"][:, 128 * h:128 * (h + 1)]),
                                     r32(xlt[:, cs:cs + 512]))
                    nc.scalar.copy(vt[:, 1024 * h + cs:1024 * h + cs + 512], ps[:])

            if debug and li == 0:
                nc.sync.dma_start(dbg["dbg_utb"][:], utb[:])
                nc.sync.dma_start(dbg["dbg_vt"][:], vt[:])

            # per sample-pair q: add + relu + reduce + softmax + attn@X
            for q in range(NPAIR if stage >= 60 else (1 if stage >= 3 else 0)):
                # T free layout (s', j, i): flat = 4096 s' + 64 j + i
                tt = []
                for h in range(2):
                    t = tbufp.tile([128, 8192], BF16, tag="T")
                    o4 = t[:].rearrange("p (s j i) -> p s j i", s=2, j=64)
                    vsl = vt[:, 1024 * h + 128 * q:1024 * h + 128 * q + 128]
                    usl = utb[:, 1024 * h + 128 * q:1024 * h + 128 * q + 128]
                    v4 = vsl.rearrange("p (s j) -> p s j", s=2).unsqueeze(3) \
                        .broadcast_to((128, 2, 64, 64))
                    u4 = usl.rearrange("p (s i) -> p s i", s=2).unsqueeze(2) \
                        .broadcast_to((128, 2, 64, 64))
                    nc.vector.tensor_tensor(o4, v4, u4, op=ALU.add)
                    nc.scalar.activation(t[:], t[:], AF.Relu)
                    tt.append(t)
                # scores [64(i), 128(s',j)]: T-chunk stationary, w2 moving
                score = score_psp.tile([64, 128], F32, tag="score")
                for h in range(2):
                    for c in range(128):
                        # PSUM zero-region: start only once per bank/tile;
                        # later writes land on pending-zero bytes (overwrite),
                        # second-half writes accumulate.
                        nc.tensor.matmul(
                            score[0:64, c:c + 1],
                            tt[h][:, 64 * c:64 * (c + 1)],
                            g["w2tb"][:, h:h + 1],
                            start=(h == 0 and c == 0), stop=(h == 1 and c == 127),
                            skip_group_check=True)

                if debug and li == 0 and q == 0:
                    ssb = smallp.tile([64, 128], F32, tag="ssb")
                    nc.vector.tensor_copy(ssb[:], score[:])
                    nc.sync.dma_start(dbg["dbg_score"][:], ssb[:])

                if stage < 4:
                    continue
                # LeakyReLU(s + b2), +mask, softmax over j per (i, s')
                att = smallp.tile([64, 128], F32, tag="att")
                nc.scalar.activation(att[:], score[:], AF.Identity,
                                     bias=g["b2bc"][0:64, 0:1])
                # LeakyReLU(x) = max(0.04*x, x)
                nc.vector.scalar_tensor_tensor(att[:], att[:], NEG_SLOPE, att[:],
                                               op0=ALU.mult, op1=ALU.max)
                nc.vector.tensor_tensor(att[:], att[:], maskt[:], op=ALU.add)
                a3 = att[:].rearrange("p (s j) -> p s j", s=2)
                mx = smallp.tile([64, 2], F32, tag="mx")
                nc.vector.tensor_reduce(mx[:], a3, axis=AX.X, op=ALU.max)
                nc.vector.tensor_tensor(
                    a3, a3, mx[:].unsqueeze(2).broadcast_to((64, 2, 64)),
                    op=ALU.subtract)
                nc.scalar.activation(att[:], att[:], AF.Exp)
                sm = smallp.tile([64, 2], F32, tag="sm")
                nc.vector.tensor_reduce(sm[:], a3, axis=AX.X, op=ALU.add)
                nc.vector.reciprocal(sm[:], sm[:])
                nc.vector.tensor_tensor(
                    a3, a3, sm[:].unsqueeze(2).broadcast_to((64, 2, 64)),
                    op=ALU.mult)

                if debug and li == 0 and q == 0:
                    nc.sync.dma_start(dbg["dbg_att"][:], att[:])

                if stage < 5:
                    continue
                # H^T [d, (s',i)] per sample; all matmul operands at
                # base partition 0 (offset-64 operands crash HW)
                hps = tr_psp.tile([128, 128], F32, tag="trps")
                for sp in range(2):
                    atT = smallp.tile([64, 64], F32, tag="atT")
                    ps = tr_psp.tile([128, 128], F32, tag="trps")
                    nc.tensor.transpose(ps[0:64, 0:64],
                                        att[:, 64 * sp:64 * sp + 64],
                                        ident[0:64, 0:64])
                    nc.vector.tensor_copy(atT[:], ps[0:64, 0:64])
                    xl = smallp.tile([64, 128], F32, tag="xl")
                    ps2 = tr_psp.tile([128, 128], F32, tag="trps")
                    nc.tensor.transpose(
                        ps2[0:64, 0:128],
                        xlt[:, 128 * q + 64 * sp:128 * q + 64 * sp + 64],
                        ident[:])
                    nc.vector.tensor_copy(xl[:], ps2[0:64, 0:128])
                    nc.tensor.matmul(hps[:, 64 * sp:64 * sp + 64],
                                     xl[:], atT[:],
                                     start=(sp == 0), stop=(sp == 1),
                                     skip_group_check=True)
                if li == 0:
                    nc.scalar.copy(h1t_all[:, 128 * q:128 * (q + 1)], hps[:])
                else:
                    outp = smallp.tile([128, 128], F32, tag="outp")
                    nc.vector.tensor_tensor(outp[:], hps[:],
                                            xh1_all[:, 128 * q:128 * (q + 1)], op=ALU.add)
                    ps2 = tr_psp.tile([128, 128], F32, tag="trps")
                    nc.tensor.transpose(ps2[:], outp[:], ident[:])
                    orows = smallp.tile([128, 128], F32, tag="orows")
                    nc.scalar.copy(orows[:], ps2[:])
                    nc.sync.dma_start(
                        out_dram[2 * q:2 * q + 2, :, :].rearrange("s n d -> (s n) d"),
                        orows[:])

            if li == 0 and stage >= 60:
                nc.vector.tensor_tensor(xh1_all[:], xt_all[:], h1t_all[:], op=ALU.add)
                if debug:
                    nc.sync.dma_start(dbg["dbg_h1t"][:], h1t_all[:])

    nc.compile()
    nc.finalize()
    return nc


_NC_CACHE = {}


def get_nc():
    if "nc" not in _NC_CACHE:
        _NC_CACHE["nc"] = build_bass()
    return _NC_CACHE["nc"]


def shard_inputs(inputs):
    in_maps = []
    for c in range(N_CORES):
        m = {}
        for k, v in inputs.items():
            v = np.asarray(v, dtype=np.float32)
            if k in ("robot_state", "human_states"):
                m[k] = np.ascontiguousarray(v[c * BC:(c + 1) * BC])
            else:
                m[k] = v
        in_maps.append(m)
    return in_maps


def run_sharded(inputs, trace=False, **kw):
    nc = get_nc()
    br = run_bass_kernel_spmd(nc, shard_inputs(inputs),
                              list(range(N_CORES)), trace=trace, **kw)
    out = np.concatenate([br.results[c]["out"] for c in range(N_CORES)], axis=0)
    return out, br


def kernel(**inputs):
    out, _ = run_sharded(inputs, trace=False)
    return out
